# revision 10
# baseline (speedup 1.0000x reference)
"""Complex multi-head attention (B=4, S=2048, D=512, H=8) on 8 TRN2 NeuronCores.

Sharding: core c handles batch b = c//2 and head group hg = c%2 (4 heads each).
Weights are head-sliced host-side; each core computes its 4 heads' attention and
a partial output projection; the host sums the two partials per batch.

v4 schedule: attention is ACT(exp)-paced (~1.15us per [128,1024] chunk); all
projection work outside the per-iteration scores/AV stream is either prologue
or "filler" matmuls emitted inside the exp-wait slack.
  - Q/K projections use the 3-multiplication Gauss complex trick (like V):
    M1=Xr@Wr, M2=Xi@Wi, M3=(Xr+Xi)@(Wr+Wi); Re=M1-M2, Im=M3-M1-M2.  Head
    group 0 (h0,h1) runs in the prologue (combines on ScE+DVE, both idle
    there); group 1 (h2,h3) runs as filler (combines on GpSimd+DVE since ScE
    streams exp).  This also cuts the weight DMA ~25% (no 2x2 real-block
    redundancy) and drops the (xr+xi) rows from the input stream -- they are
    computed on GpSimd from the re/im chunks.
  - V's M1 drain copies run on GpSimd, NOT ScE: anything on the scalar queue
    ahead of the first exp delays the whole ACT stream (queue order, not data
    deps, gated the v3 kernel's first exp by ~25us).
  - softmax denominator tree-summed on DVE down to one [128,1024] tile, then
    2 ones-matmuls broadcast-reduce it; out DMAs ride sync/gpsimd queues.
All matmuls bf16 with f32 PSUM accumulation; output stored bf16 (partials
summed in f32 on host). exp without max subtraction (|scores| <= ~18).
"""

import os

import numpy as np

import concourse.mybir as mybir
import concourse.tile as tile
from concourse import bacc
from concourse.bass import ds, ts
from concourse.bass_utils import run_bass_kernel_spmd

F32 = mybir.dt.float32
BF16 = mybir.dt.bfloat16

B, S, D = 4, 2048, 512
H, Dh = 8, 64
HPC = 4          # heads per core
SCALE = 1.0 / 8.0  # 1/sqrt(Dh)

_NC = None


def _build():
    nc = bacc.Bacc("TRN2", target_bir_lowering=False, debug=False, num_devices=8)

    # xt chunks 0-3: x.T.re rows; 4-7: x.T.im rows (contraction 1024 total)
    xt_d = nc.declare_dram_parameter("xt", [128, 8, S], BF16, isOutput=False)
    # Gauss weights [f_local, cc, m in (re, im, re+im), h*64+j] for q, k, v
    wqg_d = nc.declare_dram_parameter("wqg", [128, 4, 3, 256], BF16, isOutput=False)
    wkg_d = nc.declare_dram_parameter("wkg", [128, 4, 3, 256], BF16, isOutput=False)
    wvg_d = nc.declare_dram_parameter("wvg", [128, 4, 3, 256], BF16, isOutput=False)
    r_d = nc.declare_dram_parameter("r", [128, HPC, 1024], BF16, isOutput=False)
    ones_d = nc.declare_dram_parameter("ones", [128, 128], BF16, isOutput=False)
    out_d = nc.declare_dram_parameter("out", [S, 1024], BF16, isOutput=True)

    Exp = mybir.ActivationFunctionType.Exp

    with tile.TileContext(nc) as tc:
        with tc.tile_pool(name="sb", bufs=1) as sb:
            ones = sb.tile([128, 128], BF16)
            xt_s = sb.tile([128, 8, S], BF16)
            xsum_s = sb.tile([128, 4, S], BF16)  # xr+xi, computed on gpsimd
            wqg_s = sb.tile([128, 4, 3, 256], BF16)
            wkg_s = sb.tile([128, 4, 3, 256], BF16)
            wvg_s = sb.tile([128, 4, 3, 256], BF16)
            r_s = sb.tile([128, HPC, 1024], BF16)
            # per-head tiles so interleaved writers (filler QK waves, late
            # normalization) never alias the tiles the attention loop reads
            qts = [sb.tile([128, S], BF16, name=f"qt{hh}") for hh in range(HPC)]
            kts = [sb.tile([128, S], BF16, name=f"kt{hh}") for hh in range(HPC)]
            v = sb.tile([128, 16, 512], BF16)  # [k%128, k//128, h*128+(re|im)*64+j]
            ots = [sb.tile([128, S], BF16, name=f"ot{hh}") for hh in range(HPC)]

            # ---- input DMAs, critical-path first.  sync carries the re-row
            # chunks (K/Q M1 food), scalar the grp0 weights then the im-row
            # chunks; gpsimd the V weights.  Head-group 1 weights + r + ones
            # are filler-phase (iter 0+ / iter 6+): last. ----
            nc.scalar.dma_start(out=wkg_s[:, :, 0, 0:128], in_=wkg_d[:, :, 0, 0:128])
            nc.sync.dma_start(out=xt_s[:, 0, :], in_=xt_d[:, 0, :])
            nc.scalar.dma_start(out=wkg_s[:, :, 1, 0:128], in_=wkg_d[:, :, 1, 0:128])
            nc.sync.dma_start(out=xt_s[:, 1, :], in_=xt_d[:, 1, :])
            nc.scalar.dma_start(out=wkg_s[:, :, 2, 0:128], in_=wkg_d[:, :, 2, 0:128])
            nc.sync.dma_start(out=xt_s[:, 2, :], in_=xt_d[:, 2, :])
            nc.scalar.dma_start(out=wqg_s[:, :, :, 0:128], in_=wqg_d[:, :, :, 0:128])
            nc.sync.dma_start(out=xt_s[:, 3, :], in_=xt_d[:, 3, :])
            nc.gpsimd.dma_start(out=wvg_s[:, :, :, :], in_=wvg_d[:, :, :, :])
            nc.scalar.dma_start(out=xt_s[:, 4, :], in_=xt_d[:, 4, :])
            nc.scalar.dma_start(out=xt_s[:, 5, :], in_=xt_d[:, 5, :])
            nc.scalar.dma_start(out=xt_s[:, 6, :], in_=xt_d[:, 6, :])
            nc.scalar.dma_start(out=xt_s[:, 7, :], in_=xt_d[:, 7, :])
            nc.sync.dma_start(out=wkg_s[:, :, :, 128:256], in_=wkg_d[:, :, :, 128:256])
            nc.sync.dma_start(out=wqg_s[:, :, :, 128:256], in_=wqg_d[:, :, :, 128:256])
            nc.sync.dma_start(out=ones[:, :], in_=ones_d[:, :])
            nc.scalar.dma_start(out=r_s[:, :, :], in_=r_d[:, :, :])

            # (xr+xi) chunks on gpsimd as the pairs land
            for p in range(4):
                nc.gpsimd.tensor_add(
                    xsum_s[:, p, :], xt_s[:, p, :], xt_s[:, 4 + p, :]
                )

            # ---- prologue: Gauss Q/K for head group 0 (h0,h1) ----
            # Bank plan (8 PSUM banks = 2 pools x 4 per-tg tiles), alternating
            # so each chain's WAR lands on drains that finished long ago:
            #   K: M1->A  M2->B  M3->A'   Q: M1->B'  M2->A''  M3->B''
            with (
                tc.tile_pool(name="pgA", bufs=1, space="PSUM") as pgA,
                tc.tile_pool(name="pgB", bufs=1, space="PSUM") as pgB,
                tc.tile_pool(name="qgs", bufs=1) as qgs,
            ):
                def gauss_grp0(wg_s, dst01, pM1, pM2, pM3, pfx):
                    # DVE lanes are partition-locked, so only the aligned
                    # quadrants (re of h-even -> rows 0:64, im of h-odd ->
                    # rows 64:128) are written in place; the crossed pair is
                    # staged at its natural partitions in sh and moved by an
                    # SBUF->SBUF DMA (DMA is free to cross partitions).
                    sh = qgs.tile([128, S], BF16, name=f"{pfx}sh")
                    m1 = {}
                    for p in range(4):
                        for tg in range(4):
                            if p == 0:
                                m1[tg] = pM1.tile([128, 512], F32, name=f"g{tg}")
                            nc.tensor.matmul(
                                m1[tg][:, :],
                                lhsT=wg_s[:, p, 0, 0:128],
                                rhs=xt_s[:, p, ts(tg, 512)],
                                start=(p == 0),
                                stop=(p == 3),
                            )
                    a1 = {}
                    for tg in range(4):
                        a1[tg] = qgs.tile([128, 512], F32, name=f"{pfx}a{tg}")
                        nc.scalar.copy(out=a1[tg][:, :], in_=m1[tg][:, :])
                    m2 = {}
                    for p in range(4):
                        for tg in range(4):
                            if p == 0:
                                m2[tg] = pM2.tile([128, 512], F32, name=f"g{tg}")
                            nc.tensor.matmul(
                                m2[tg][:, :],
                                lhsT=wg_s[:, p, 1, 0:128],
                                rhs=xt_s[:, 4 + p, ts(tg, 512)],
                                start=(p == 0),
                                stop=(p == 3),
                            )
                    tmp = {}
                    for tg in range(4):
                        nc.vector.tensor_sub(
                            dst01[0][ds(0, 64), ts(tg, 512)],
                            a1[tg][ds(0, 64), :],
                            m2[tg][ds(0, 64), :],
                        )
                        nc.vector.tensor_sub(
                            sh[ds(64, 64), ts(tg, 512)],
                            a1[tg][ds(64, 64), :],
                            m2[tg][ds(64, 64), :],
                        )
                        tmp[tg] = qgs.tile([128, 512], F32, name=f"{pfx}t{tg}")
                        nc.vector.tensor_add(tmp[tg][:, :], a1[tg][:, :], m2[tg][:, :])
                    m3 = {}
                    for p in range(4):
                        for tg in range(4):
                            if p == 0:
                                m3[tg] = pM3.tile([128, 512], F32, name=f"g{tg}")
                            nc.tensor.matmul(
                                m3[tg][:, :],
                                lhsT=wg_s[:, p, 2, 0:128],
                                rhs=xsum_s[:, p, ts(tg, 512)],
                                start=(p == 0),
                                stop=(p == 3),
                            )
                    for tg in range(4):
                        nc.vector.tensor_sub(
                            dst01[1][ds(64, 64), ts(tg, 512)],
                            m3[tg][ds(64, 64), :],
                            tmp[tg][ds(64, 64), :],
                        )
                        nc.vector.tensor_sub(
                            sh[ds(0, 64), ts(tg, 512)],
                            m3[tg][ds(0, 64), :],
                            tmp[tg][ds(0, 64), :],
                        )
                    # re of h-odd / im of h-even move to their head tiles
                    nc.sync.dma_start(out=dst01[1][ds(0, 64), :], in_=sh[ds(64, 64), :])
                    nc.sync.dma_start(out=dst01[0][ds(64, 64), :], in_=sh[ds(0, 64), :])

                gauss_grp0(wkg_s, (kts[0], kts[1]), pgA, pgB, pgA, "k")
                gauss_grp0(wqg_s, (qts[0], qts[1]), pgB, pgA, pgB, "q")

            # ---- V projection, Gauss 3-mult (M1 drains on gpsimd: the
            # scalar queue must stay clear ahead of the first exp) ----
            with (
                tc.tile_pool(name="vg", bufs=2, space="PSUM") as vgp,
                tc.tile_pool(name="vt", bufs=2) as vtp,
            ):
                for tb in range(16):
                    ms = [vgp.tile([128, 512], F32, name=f"m{mi}", tag=f"m{mi}")
                          for mi in range(3)]
                    for mi in range(3):
                        src = xsum_s if mi == 2 else xt_s
                        for cc in range(4):
                            c = cc + (0, 4, 0)[mi]
                            nc.tensor.matmul(
                                ms[mi][:, 0:256],
                                lhsT=src[:, c, ts(tb, 128)],
                                rhs=wvg_s[:, cc, mi, :],
                                start=(cc == 0),
                                stop=(cc == 3),
                            )
                    # DVE has 2 SBUF read ports: keep <=1 PSUM operand
                    # per op.  a1 = M1 (SBUF), tmp = M1+M2 (SBUF), then
                    # Re = a1 - M2 and Im = M3 - tmp per head.
                    a1 = vtp.tile([128, 256], F32, name="a1")
                    tmp = vtp.tile([128, 256], F32, name="tmp")
                    # ScE is idle during the prologue (gpsimd has no PSUM port)
                    nc.scalar.copy(out=a1[:, :], in_=ms[0][:, 0:256])
                    nc.vector.tensor_add(tmp[:, :], a1[:, :], ms[1][:, 0:256])
                    for hh in range(HPC):
                        nc.vector.tensor_sub(
                            v[:, tb, ds(hh * 128, 64)],
                            a1[:, ds(hh * 64, 64)],
                            ms[1][:, ds(hh * 64, 64)],
                        )
                        nc.vector.tensor_sub(
                            v[:, tb, ds(hh * 128 + 64, 64)],
                            ms[2][:, ds(hh * 64, 64)],
                            tmp[:, ds(hh * 64, 64)],
                        )

            # ---- attention + interleaved filler (Gauss QK grp1 + out proj) ----
            with (
                tc.tile_pool(name="st", bufs=2, space="PSUM") as stp,
                tc.tile_pool(name="ov", bufs=1, space="PSUM") as ovp,
                tc.tile_pool(name="fq", bufs=2, space="PSUM") as fqp,
                tc.tile_pool(name="pt", bufs=4) as ptp,
                tc.tile_pool(name="pr", bufs=3) as prp,
                tc.tile_pool(name="qd", bufs=3) as qdp,
                tc.tile_pool(name="misc", bufs=2) as miscp,
                tc.tile_pool(name="or", bufs=2) as orp,
                tc.tile_pool(name="ysb", bufs=3) as ysb,
                tc.tile_pool(name="fsb", bufs=2) as fsbp,
            ):
                # filler units: each closure emits ONE PE matmul (plus the
                # combine ops that hang off chain completions).
                filler = []

                def qk_gauss_proj_units(wg_s, dst23):
                    # grp1 (h2,h3), whole projection: per tg, M1 -> qkf ring
                    # slot a, M2 -> slot b, M3 -> slot a again (WAR waits only
                    # the gpsimd a1-copy).  Crossed quadrants stage in sh; one
                    # DMA pair per projection moves them after tg3.
                    shbox = {}
                    units = []

                    def mk_tg(tg):
                        box = {}

                        def m1(p):
                            if p == 0:
                                if tg == 0:
                                    shbox["sh"] = fsbp.tile([128, S], BF16, name="fsh")
                                box["m1"] = fqp.tile([128, 512], F32, name="qkf")
                            nc.tensor.matmul(
                                box["m1"][:, :],
                                lhsT=wg_s[:, p, 0, 128:256],
                                rhs=xt_s[:, p, ts(tg, 512)],
                                start=(p == 0),
                                stop=(p == 3),
                            )
                            if p == 3:
                                # DVE, not ScE (exp stream) nor gpsimd (no
                                # PSUM port)
                                box["a1"] = fsbp.tile([128, 512], F32, name="fa1")
                                nc.vector.tensor_copy(
                                    out=box["a1"][:, :], in_=box["m1"][:, :]
                                )

                        def m2(p):
                            if p == 0:
                                box["m2"] = fqp.tile([128, 512], F32, name="qkf")
                            nc.tensor.matmul(
                                box["m2"][:, :],
                                lhsT=wg_s[:, p, 1, 128:256],
                                rhs=xt_s[:, 4 + p, ts(tg, 512)],
                                start=(p == 0),
                                stop=(p == 3),
                            )
                            if p == 3:
                                nc.vector.tensor_sub(
                                    dst23[0][ds(0, 64), ts(tg, 512)],
                                    box["a1"][ds(0, 64), :],
                                    box["m2"][ds(0, 64), :],
                                )
                                nc.vector.tensor_sub(
                                    shbox["sh"][ds(64, 64), ts(tg, 512)],
                                    box["a1"][ds(64, 64), :],
                                    box["m2"][ds(64, 64), :],
                                )
                                box["tmp"] = fsbp.tile([128, 512], F32, name="ftmp")
                                nc.vector.tensor_add(
                                    box["tmp"][:, :], box["a1"][:, :], box["m2"][:, :]
                                )

                        def m3(p):
                            if p == 0:
                                box["m3"] = fqp.tile([128, 512], F32, name="qkf")
                            nc.tensor.matmul(
                                box["m3"][:, :],
                                lhsT=wg_s[:, p, 2, 128:256],
                                rhs=xsum_s[:, p, ts(tg, 512)],
                                start=(p == 0),
                                stop=(p == 3),
                            )
                            if p == 3:
                                nc.vector.tensor_sub(
                                    dst23[1][ds(64, 64), ts(tg, 512)],
                                    box["m3"][ds(64, 64), :],
                                    box["tmp"][ds(64, 64), :],
                                )
                                nc.vector.tensor_sub(
                                    shbox["sh"][ds(0, 64), ts(tg, 512)],
                                    box["m3"][ds(0, 64), :],
                                    box["tmp"][ds(0, 64), :],
                                )
                                if tg == 3:
                                    sh = shbox["sh"]
                                    nc.sync.dma_start(
                                        out=dst23[1][ds(0, 64), :], in_=sh[ds(64, 64), :]
                                    )
                                    nc.gpsimd.dma_start(
                                        out=dst23[0][ds(64, 64), :], in_=sh[ds(0, 64), :]
                                    )

                        return (
                            [lambda p=p: m1(p) for p in range(4)]
                            + [lambda p=p: m2(p) for p in range(4)]
                            + [lambda p=p: m3(p) for p in range(4)]
                        )

                    for tg in range(4):
                        units.extend(mk_tg(tg))
                    return units

                def o_tb_units(tb):
                    box = {}

                    def mm(g, hc):
                        if g == 0 and hc == 0:
                            box[0] = fqp.tile([128, 512], F32, name="qkf")
                            box[1] = fqp.tile([128, 512], F32, name="qkf")
                        nc.tensor.matmul(
                            box[g][:, :],
                            lhsT=ots[hc][:, ts(tb, 128)],
                            rhs=r_s[:, hc, ts(g, 512)],
                            start=(hc == 0),
                            stop=(hc == 3),
                        )
                        if g == 1 and hc == 3:
                            y_s = ysb.tile([128, 1024], BF16)
                            nc.vector.tensor_copy(out=y_s[:, 0:512], in_=box[0][:, :])
                            nc.vector.tensor_copy(out=y_s[:, 512:1024], in_=box[1][:, :])
                            # keep the scalar queue exp-only during attention
                            q_ = nc.sync if tb % 2 == 0 else nc.gpsimd
                            q_.dma_start(out=out_d[ts(tb, 128), :], in_=y_s[:, :])

                    return [lambda g=g, hc=hc: mm(g, hc) for g in range(2) for hc in range(HPC)]

                # deadline order: kts/qts grp1 needed at iter 4 (h2) / 5 (h3)
                filler.extend(qk_gauss_proj_units(wkg_s, (kts[2], kts[3])))
                filler.extend(qk_gauss_proj_units(wqg_s, (qts[2], qts[3])))

                iters = [(0, 0), (0, 1), (1, 0), (1, 1), (2, 0), (3, 0), (2, 1), (3, 1)]

                def fill_n(it, kc):
                    # Gauss QK grp1 (96 units) over iters 0-3 (112 slots);
                    # out projection tb0-7 (64 units) over iter-6 kc>=4 +
                    # iter 7 (iter-6 kc<4 must stay empty: those units read
                    # ot written by norms only emitted at kc==3).  Cap near 2
                    # per chunk so PE never outruns the exp stream for long.
                    if it < 4:
                        return 2 if (kc % 4) != 3 else 1
                    if it in (4, 5):
                        return 0
                    if it == 6:
                        return 0 if kc < 4 else 2
                    return 3 if kc < 8 else 2
                pending = [None]
                fi = 0
                for it, (h, qh) in enumerate(iters):
                    if it == 6:
                        for tb in range(8):
                            filler.extend(o_tb_units(tb))
                    o_halves = (
                        ovp.tile([128, 512], F32, name="o0", tag="o0"),
                        ovp.tile([128, 512], F32, name="o1", tag="o1"),
                    )
                    pts, pairs = [], []
                    oraw = orp.tile([128, 1024], BF16, name="oraw")
                    def emit_scores(kc):
                        st_t = stp.tile([128, 1024], F32, name="st_t")
                        for g in range(2):
                            nc.tensor.matmul(
                                st_t[:, ts(g, 512)],
                                lhsT=kts[h][:, ts(kc, 128)],
                                rhs=qts[h][:, ds(qh * 1024 + g * 512, 512)],
                                start=True,
                                stop=True,
                            )
                        pt_t = ptp.tile([128, 1024], BF16)
                        nc.scalar.activation(
                            out=pt_t[:, :], in_=st_t[:, :], func=Exp, scale=SCALE
                        )
                        pts.append(pt_t)

                    emit_scores(0)
                    for kc in range(16):
                        # scores for the NEXT chunk go ahead of this chunk's
                        # exp-dependent AV matmuls (keeps ACT streaming and
                        # gives the PE queue work while exp(kc) runs).
                        if kc + 1 < 16:
                            emit_scores(kc + 1)
                        for _ in range(fill_n(it, kc)):
                            if fi < len(filler):
                                filler[fi]()
                                fi += 1
                        if kc == 3 and pending[0] is not None:
                            pending[0]()
                            pending[0] = None
                        pt_t = pts[kc]
                        for g in range(2):
                            nc.tensor.matmul(
                                o_halves[g][:, :],
                                lhsT=v[:, kc, ds(h * 128, 128)],
                                rhs=pt_t[:, ts(g, 512)],
                                start=(kc == 0),
                                stop=(kc == 15),
                            )
                        if kc == 15:
                            # drain o ahead of the last tree adds: the next
                            # iteration's first AV then only waits on these
                            # two casts, not on the d/recip/mul chain.
                            nc.vector.tensor_copy(
                                out=oraw[:, 0:512], in_=o_halves[0][:, :]
                            )
                            nc.vector.tensor_copy(
                                out=oraw[:, 512:1024], in_=o_halves[1][:, :]
                            )
                        # denominator: pair-sum then running-sum on the
                        # Vector engine -- the final sum is ready one add
                        # after the last exp (short cross-engine tail).
                        if kc % 2 == 1:
                            pr = prp.tile([128, 1024], BF16)
                            nc.vector.tensor_add(pr[:, :], pts[kc - 1][:, :], pts[kc][:, :])
                            if not pairs:
                                pairs.append(pr)
                            else:
                                rn = qdp.tile([128, 1024], BF16, name="run")
                                nc.vector.tensor_add(
                                    rn[:, :], pairs[-1][:, :], pr[:, :]
                                )
                                pairs.append(rn)
                    fin = pairs[-1]

                    def norm(h=h, qh=qh, fin=fin, oraw=oraw):
                        # deferred: emitted a few chunks into the NEXT
                        # iteration so the d->recip->mul chain never stalls
                        # the tensor engine.
                        d_t = stp.tile([128, 1024], F32, name="st_t")
                        for g in range(2):
                            nc.tensor.matmul(
                                d_t[:, ts(g, 512)],
                                lhsT=ones[:, :],
                                rhs=fin[:, ts(g, 512)],
                                start=True,
                                stop=True,
                            )
                        rec = miscp.tile([128, 1024], F32)
                        nc.vector.reciprocal_approx_fast(out=rec[:, :], in_=d_t[:, :])
                        for g in range(2):
                            nc.vector.tensor_mul(
                                ots[h][:, ds(qh * 1024 + g * 512, 512)],
                                oraw[:, ts(g, 512)],
                                rec[:, ts(g, 512)],
                            )

                    pending[0] = norm
                pending[0]()
                pending[0] = None
                # drain leftover filler (none expected), then tail: out proj
                # for token blocks 8-15.
                while fi < len(filler):
                    filler[fi]()
                    fi += 1
                for tb in range(8, 16):
                    for u in o_tb_units(tb):
                        u()

    nc.compile()
    return nc


def _core_inputs(x, wq, wk, wv, wo, core):
    import ml_dtypes

    b, hg = divmod(core, 2)
    heads = [hg * HPC + h for h in range(HPC)]

    xr = x[b].T.real.astype(np.float32)   # [512, 2048]
    xi = x[b].T.imag.astype(np.float32)
    xt = np.concatenate([xr, xi], axis=0)  # [1024, 2048]
    xt = np.ascontiguousarray(xt.reshape(8, 128, S).transpose(1, 0, 2))

    def _wg(w):
        # Gauss layout [128, cc, m in (re, im, re+im), h*64+j]
        wr = np.concatenate(
            [w[gh * Dh : (gh + 1) * Dh].real.T.astype(np.float32) for gh in heads],
            axis=1,
        )  # [512, 256]
        wi = np.concatenate(
            [w[gh * Dh : (gh + 1) * Dh].imag.T.astype(np.float32) for gh in heads],
            axis=1,
        )
        wg = np.stack([wr, wi, wr + wi], axis=1)  # [512, 3, 256]
        return np.ascontiguousarray(
            wg.reshape(4, 128, 3, 256).transpose(1, 0, 2, 3)
        )  # [128, 4, 3, 256]

    r_blocks = []
    for gh in heads:
        wo_h = wo[:, gh * Dh : (gh + 1) * Dh]  # [512, 64] complex
        wor = np.ascontiguousarray(wo_h.real).astype(np.float32)
        woi = np.ascontiguousarray(wo_h.imag).astype(np.float32)
        top = np.concatenate([wor.T, woi.T], axis=1)    # O_re rows -> [64, 1024]
        bot = np.concatenate([-woi.T, wor.T], axis=1)   # O_im rows
        r_blocks.append(np.concatenate([top, bot], axis=0))  # [128, 1024]
    r_cat = np.concatenate(r_blocks, axis=0)  # [512, 1024]
    r_cat = np.ascontiguousarray(r_cat.reshape(HPC, 128, 1024).transpose(1, 0, 2))

    out = {
        "xt": xt,
        "wqg": _wg(wq),
        "wkg": _wg(wk),
        "wvg": _wg(wv),
        "r": r_cat,
        "ones": np.ones((128, 128), dtype=np.float32),
    }
    return {k: v.astype(ml_dtypes.bfloat16) for k, v in out.items()}


def kernel(x, wq, wk, wv, wo):
    global _NC
    x = np.asarray(x)
    wq = np.asarray(wq)
    wk = np.asarray(wk)
    wv = np.asarray(wv)
    wo = np.asarray(wo)

    if _NC is None:
        _NC = _build()

    in_maps = [_core_inputs(x, wq, wk, wv, wo, c) for c in range(8)]

    trace = os.environ.get("KERNEL_PROFILE", "0") == "1"
    kwargs = {}
    if trace:
        _install_profile_shim()
        kwargs = {"trace": True}
    res = run_bass_kernel_spmd(_NC, in_maps, core_ids=list(range(8)), **kwargs)
    if trace:
        print(f"HW exec time: {res.exec_time_ns} ns")

    out = np.zeros((B, S, D), dtype=np.complex64)
    for c in range(8):
        b = c // 2
        y = np.asarray(res.results[c]["out"]).astype(np.float32)
        out[b] += y[:, :512] + 1j * y[:, 512:]
    return out


def _install_profile_shim():
    """Register the NTFF profile hook for axon (missing antenv.axon_hooks)."""
    import contextlib
    import ctypes
    import sys
    import types

    try:
        import antenv.axon_hooks  # noqa: F401

        return
    except ImportError:
        pass

    so_path = "/opt/axon/libaxon_pjrt.so"
    lib = ctypes.CDLL(so_path)
    if not hasattr(lib, "axon_start_nrt_profile"):
        return
    lib.axon_start_nrt_profile.argtypes = [
        ctypes.POINTER(ctypes.c_int64),
        ctypes.c_size_t,
    ]
    lib.axon_start_nrt_profile.restype = ctypes.c_int64
    lib.axon_stop_nrt_profile.argtypes = [ctypes.c_char_p]
    lib.axon_stop_nrt_profile.restype = ctypes.c_int64

    @contextlib.contextmanager
    def _hook(output_dir, device_ids):
        import jax

        jax.devices()
        if device_ids:
            ids = (ctypes.c_int64 * len(device_ids))(*device_ids)
            rc = lib.axon_start_nrt_profile(ids, len(device_ids))
        else:
            rc = lib.axon_start_nrt_profile(None, 0)
        if rc != 0:
            raise RuntimeError(f"axon_start_nrt_profile rc={rc}")
        try:
            yield
        finally:
            n = lib.axon_stop_nrt_profile(str(output_dir).encode())
            print(f"profile: {n} file(s) -> {output_dir}", file=sys.stderr)

    mod = types.ModuleType("antenv.axon_hooks")
    _h = [_hook]

    mod.set_axon_ntff_profile_hook = lambda h: _h.__setitem__(0, h)
    mod.get_axon_ntff_profile_hook = lambda: _h[0]
    sys.modules["antenv.axon_hooks"] = mod
    import antenv

    antenv.axon_hooks = mod

    import concourse.bass_utils as bu

    bu.upload_artifacts = lambda tmpdir: str(tmpdir)
